# revision 5
# baseline (speedup 1.0000x reference)
"""Bucket-indexed spatially-varying (channel-shared) 5x5 convolution on 8 trn2 cores.

out[b,c,y,x] = sum_{i,j} pad(input)[b,c,y+i,x+j] * kernel_bank[buckets[b,y,x], i, j]

Strategy (data-parallel over batch, one image per core), all bf16 on device:
  * Phase A: buckets -> one-hot (DVE is_equal) -> PE matmul against the
    [64,25] bank -> per-pixel weight map wm staged to DRAM as [y, tap, x].
  * Phase B layout: partition = output row y (2 chunks of 128), free =
    (c, x) with x minor. Five row-shifted copies of the input tile make
    the dy shift a partition-aligned read; dx is a free-dim offset.
  * Per tap: ONE big DVE tensor_tensor mult (weight map broadcast across
    c via a stride-0 AP; x-minor keeps every operand packed bf16 so the
    DVE 2x perf mode engages):
        P[y, c, x] = xs[y+i, c, x+j] * wm[y, t, x]
    The 25 tap products are accumulated on the otherwise-idle PE with
    identity-stationary matmuls into PSUM (fp32), then evicted to bf16
    by the ACT engine. Host converts bf16 -> fp32.
"""

import sys

sys.path.insert(0, "/opt/trn_rl_repo")

import numpy as np

B, C, H, W = 8, 128, 256, 256
K, NB = 5, 64
PAD = (K - 1) // 2  # 2
HP, WP = H + 2 * PAD, W + 2 * PAD  # 260, 260
N_CORES = 8
NT = K * K  # 25 taps

CBLK = 16  # channel block
NCB = C // CBLK  # 8
XH = 128  # x half width
NXH = W // XH  # 2

_CACHE = {}


def _build_nc():
    import concourse.bacc as bacc
    import concourse.mybir as mybir
    from concourse import tile

    f32 = mybir.dt.float32
    bf16 = mybir.dt.bfloat16
    Alu = mybir.AluOpType
    Act = mybir.ActivationFunctionType

    nc = bacc.Bacc(None)

    # channel-mid layout [row, c, x]: per-partition contiguous c-block rows
    xp = nc.dram_tensor("xp", [HP, C, WP], bf16, kind="ExternalInput")
    bkf = nc.dram_tensor("bkf", [H, W], bf16, kind="ExternalInput")
    bank = nc.dram_tensor("bank", [NB, NT], bf16, kind="ExternalInput")
    iota = nc.dram_tensor("iota", [NB, 1], f32, kind="ExternalInput")
    ident = nc.dram_tensor("ident", [128, 128], bf16, kind="ExternalInput")
    y_out = nc.dram_tensor("y", [H, C, W], bf16, kind="ExternalOutput")

    GROWS = 8  # bucket rows per wm-build group
    GPIX = GROWS * W  # 2048
    GPC = 128 // GROWS  # 16 groups per y chunk

    with tile.TileContext(nc) as tc:
        with tc.tile_pool(name="dram", bufs=1, space="DRAM") as dpool:
            # weight map staged in DRAM as [y, tap, x]
            wm_dram = dpool.tile([H, NT, W], bf16)

            with (
                tc.tile_pool(name="const", bufs=1) as kpool,
                tc.tile_pool(name="wbuild", bufs=2) as wpool,
                tc.tile_pool(name="wm", bufs=2) as wmpool,
                tc.tile_pool(name="xs", bufs=2) as xpool,
                tc.tile_pool(name="prod", bufs=3) as ppool,
                tc.tile_pool(name="out", bufs=2) as opool,
                tc.tile_pool(name="psum", bufs=2, space="PSUM") as pspool,
            ):
                bank_sb = kpool.tile([NB, NT], bf16)
                nc.sync.dma_start(out=bank_sb[:], in_=bank[:])
                iota_sb = kpool.tile([NB, 1], f32)
                nc.sync.dma_start(out=iota_sb[:], in_=iota[:])
                ident_sb = kpool.tile([128, 128], bf16)
                nc.sync.dma_start(out=ident_sb[:], in_=ident[:])

                def wm_group(g):
                    # one-hot + PE matmul against the bank -> wm rows
                    brep = wpool.tile([NB, GPIX], bf16, tag="brep")
                    nc.sync.dma_start(
                        out=brep[:],
                        in_=bkf[g * GROWS : (g + 1) * GROWS, :]
                        .rearrange("(o h) w -> o (h w)", o=1)
                        .broadcast_to((NB, GPIX)),
                    )
                    oh = wpool.tile([NB, GPIX], bf16, tag="oh")
                    nc.vector.tensor_scalar(
                        out=oh[:],
                        in0=brep[:],
                        scalar1=iota_sb[:],
                        scalar2=None,
                        op0=Alu.is_equal,
                    )
                    # borrow the conv psum buffers (same tag/shape)
                    ps = pspool.tile([128, CBLK * XH], f32, tag="acc")
                    for s in range(GPIX // 512):
                        nc.tensor.matmul(
                            ps[0:NT, s * 512 : (s + 1) * 512],
                            bank_sb[:],
                            oh[:, s * 512 : (s + 1) * 512],
                            start=True,
                            stop=True,
                        )
                    wms = wpool.tile([NT, GPIX], bf16, tag="wms")
                    nc.scalar.copy(out=wms[:], in_=ps[0:NT, 0:GPIX])
                    y0 = g * GROWS
                    # keep the SBUF partition dim (t) first on both sides;
                    # an SBUF-side rearrange that moves the partition dim
                    # scrambles the transfer.
                    nc.sync.dma_start(
                        out=wm_dram[y0 : y0 + GROWS, :, :].rearrange(
                            "y t x -> t y x"
                        ),
                        in_=wms.rearrange("t (y x) -> t y x", y=GROWS),
                    )

                # chunk 0's weight map as a prefix; chunk 1's is
                # interleaved into chunk 0's conv loop below
                for g in range(GPC):
                    wm_group(g)

                for a in (0, 128):  # y chunk
                    wt = wmpool.tile([128, NT, W], bf16, tag="wt")
                    nc.sync.dma_start(
                        out=wt[:], in_=wm_dram[a : a + 128, :, :]
                    )
                    for cb in range(NCB):
                        c0 = cb * CBLK
                        xts = []
                        for i in range(K):
                            xt = xpool.tile(
                                [128, CBLK, WP], bf16, tag=f"xt{i}"
                            )
                            nc.sync.dma_start(
                                out=xt[:],
                                in_=xp[
                                    a + i : a + i + 128,
                                    c0 : c0 + CBLK,
                                    :,
                                ],
                            )
                            xts.append(xt)
                        for xh in range(NXH):
                            x0 = xh * XH
                            acc = pspool.tile([128, CBLK * XH], f32, tag="acc")
                            for t in range(NT):
                                i, j = t // K, t % K
                                p = ppool.tile([128, CBLK, XH], bf16, tag="p")
                                nc.vector.tensor_tensor(
                                    out=p[:],
                                    in0=xts[i][:, :, x0 + j : x0 + j + XH],
                                    in1=wt[:, t, x0 : x0 + XH]
                                    .unsqueeze(1)
                                    .broadcast_to((128, CBLK, XH)),
                                    op=Alu.mult,
                                )
                                pf = p.rearrange("p c x -> p (c x)")
                                for s in range(CBLK * XH // 512):
                                    nc.tensor.matmul(
                                        acc[:, s * 512 : (s + 1) * 512],
                                        ident_sb[:],
                                        pf[:, s * 512 : (s + 1) * 512],
                                        start=(t == 0),
                                        stop=(t == NT - 1),
                                    )
                            ot = opool.tile([128, CBLK * XH], bf16, tag="ot")
                            nc.scalar.copy(out=ot[:], in_=acc[:])
                            nc.sync.dma_start(
                                out=y_out[
                                    a : a + 128, c0 : c0 + CBLK, x0 : x0 + XH
                                ],
                                in_=ot.rearrange("p (c x) -> p c x", c=CBLK),
                            )
                        if a == 0:
                            # slip chunk 1's wm build under chunk 0's conv
                            wm_group(GPC + 2 * cb)
                            wm_group(GPC + 2 * cb + 1)

    nc.finalize()
    return nc


def _get_nc():
    if "nc" not in _CACHE:
        _CACHE["nc"] = _build_nc()
    return _CACHE["nc"]


def _make_in_maps(inputs):
    import concourse.mybir as mybir

    bf16 = mybir.dt.np(mybir.dt.bfloat16)

    x = np.ascontiguousarray(inputs["input"], dtype=np.float32)
    # pad spatially, then [b, row, c, x] channel-mid layout
    xpad = np.pad(x, ((0, 0), (0, 0), (PAD, PAD), (PAD, PAD)))
    xpad = np.ascontiguousarray(xpad.transpose(0, 2, 1, 3)).astype(bf16)
    bkf = np.ascontiguousarray(inputs["buckets"], dtype=np.int32).astype(
        np.float32
    ).astype(bf16)  # ids < 64: exact in bf16
    bank2 = (
        np.ascontiguousarray(inputs["kernel_bank"], dtype=np.float32)
        .reshape(NB, NT)
        .astype(bf16)
    )
    iota64 = np.arange(NB, dtype=np.float32).reshape(NB, 1)
    ident = np.eye(128, dtype=np.float32).astype(bf16)
    return [
        {
            "xp": xpad[i],
            "bkf": bkf[i],
            "bank": bank2,
            "iota": iota64,
            "ident": ident,
        }
        for i in range(N_CORES)
    ]


def kernel(input, kernel_bank, buckets):
    from concourse.bass_utils import run_bass_kernel_spmd

    nc = _get_nc()
    in_maps = _make_in_maps(
        {"input": input, "kernel_bank": kernel_bank, "buckets": buckets}
    )
    res = run_bass_kernel_spmd(nc, in_maps, list(range(N_CORES)))
    # device output is [H, C, W] bf16; back to [C, H, W] fp32
    out = np.stack(
        [
            res.results[i]["y"].astype(np.float32).transpose(1, 0, 2)
            for i in range(N_CORES)
        ],
        axis=0,
    )
    return np.ascontiguousarray(out, dtype=np.float32)


# revision 7
# speedup vs baseline: 1.0295x; 1.0295x over previous
"""Bucket-indexed spatially-varying (channel-shared) 5x5 convolution on 8 trn2 cores.

out[b,c,y,x] = sum_{i,j} pad(input)[b,c,y+i,x+j] * kernel_bank[buckets[b,y,x], i, j]

Strategy (data-parallel over batch, one image per core), all bf16 on device:
  * Phase A: buckets -> one-hot (DVE is_equal) -> PE matmul against the
    [64,25] bank -> per-pixel weight map wm staged to DRAM as [y, tap, x].
  * Phase B layout: partition = output row y (2 chunks of 128), free =
    (c, x) with x minor. Five row-shifted copies of the input tile make
    the dy shift a partition-aligned read; dx is a free-dim offset.
  * Per tap: ONE big DVE tensor_tensor mult (weight map broadcast across
    c via a stride-0 AP; x-minor keeps every operand packed bf16 so the
    DVE 2x perf mode engages):
        P[y, c, x] = xs[y+i, c, x+j] * wm[y, t, x]
    The 25 tap products are accumulated on the otherwise-idle PE with
    identity-stationary matmuls into PSUM (fp32), then evicted to bf16
    by the ACT engine. Host converts bf16 -> fp32.
"""

import sys

sys.path.insert(0, "/opt/trn_rl_repo")

import numpy as np

B, C, H, W = 8, 128, 256, 256
K, NB = 5, 64
PAD = (K - 1) // 2  # 2
HP, WP = H + 2 * PAD, W + 2 * PAD  # 260, 260
N_CORES = 8
NT = K * K  # 25 taps

CBLK = 16  # channel block
NCB = C // CBLK  # 8
XH = 128  # x half width
NXH = W // XH  # 2

_CACHE = {}


def _build_nc():
    import concourse.bacc as bacc
    import concourse.mybir as mybir
    from concourse import tile

    f32 = mybir.dt.float32
    bf16 = mybir.dt.bfloat16
    Alu = mybir.AluOpType
    Act = mybir.ActivationFunctionType

    nc = bacc.Bacc(None)

    # channel-mid layout [row, c, x]: per-partition contiguous c-block rows
    xp = nc.dram_tensor("xp", [HP, C, WP], bf16, kind="ExternalInput")
    bkf = nc.dram_tensor("bkf", [H, W], bf16, kind="ExternalInput")
    bank = nc.dram_tensor("bank", [NB, NT], bf16, kind="ExternalInput")
    iota = nc.dram_tensor("iota", [NB, 1], f32, kind="ExternalInput")
    ident = nc.dram_tensor("ident", [128, 128], bf16, kind="ExternalInput")
    y_out = nc.dram_tensor("y", [H, C, W], bf16, kind="ExternalOutput")

    GROWS = 8  # bucket rows per wm-build group
    GPIX = GROWS * W  # 2048
    GPC = 128 // GROWS  # 16 groups per y chunk

    with tile.TileContext(nc) as tc:
        with tc.tile_pool(name="dram", bufs=1, space="DRAM") as dpool:
            # weight map staged in DRAM as [y, tap, x]
            wm_dram = dpool.tile([H, NT, W], bf16)

            with (
                tc.tile_pool(name="const", bufs=1) as kpool,
                tc.tile_pool(name="wbuild", bufs=2) as wpool,
                tc.tile_pool(name="wm", bufs=2) as wmpool,
                tc.tile_pool(name="xs", bufs=2) as xpool,
                tc.tile_pool(name="prod", bufs=4) as ppool,
                tc.tile_pool(name="gacc", bufs=2) as gpool,
                tc.tile_pool(name="out", bufs=2) as opool,
                tc.tile_pool(name="psum", bufs=2, space="PSUM") as pspool,
            ):
                bank_sb = kpool.tile([NB, NT], bf16)
                nc.sync.dma_start(out=bank_sb[:], in_=bank[:])
                iota_sb = kpool.tile([NB, 1], f32)
                nc.sync.dma_start(out=iota_sb[:], in_=iota[:])
                ident_sb = kpool.tile([128, 128], bf16)
                nc.sync.dma_start(out=ident_sb[:], in_=ident[:])

                def wm_group(g):
                    # one-hot + PE matmul against the bank -> wm rows
                    brep = wpool.tile([NB, GPIX], bf16, tag="brep")
                    nc.sync.dma_start(
                        out=brep[:],
                        in_=bkf[g * GROWS : (g + 1) * GROWS, :]
                        .rearrange("(o h) w -> o (h w)", o=1)
                        .broadcast_to((NB, GPIX)),
                    )
                    oh = wpool.tile([NB, GPIX], bf16, tag="oh")
                    nc.vector.tensor_scalar(
                        out=oh[:],
                        in0=brep[:],
                        scalar1=iota_sb[:],
                        scalar2=None,
                        op0=Alu.is_equal,
                    )
                    # borrow the conv psum buffers (same tag/shape)
                    ps = pspool.tile([128, CBLK * XH], f32, tag="acc")
                    for s in range(GPIX // 512):
                        nc.tensor.matmul(
                            ps[0:NT, s * 512 : (s + 1) * 512],
                            bank_sb[:],
                            oh[:, s * 512 : (s + 1) * 512],
                            start=True,
                            stop=True,
                        )
                    wms = wpool.tile([NT, GPIX], bf16, tag="wms")
                    nc.scalar.copy(out=wms[:], in_=ps[0:NT, 0:GPIX])
                    y0 = g * GROWS
                    # keep the SBUF partition dim (t) first on both sides;
                    # an SBUF-side rearrange that moves the partition dim
                    # scrambles the transfer.
                    nc.sync.dma_start(
                        out=wm_dram[y0 : y0 + GROWS, :, :].rearrange(
                            "y t x -> t y x"
                        ),
                        in_=wms.rearrange("t (y x) -> t y x", y=GROWS),
                    )

                # chunk 0's weight map as a prefix; chunk 1's is
                # interleaved into chunk 0's conv loop below
                for g in range(GPC):
                    wm_group(g)

                for a in (0, 128):  # y chunk
                    wt = wmpool.tile([128, NT, W], bf16, tag="wt")
                    nc.sync.dma_start(
                        out=wt[:], in_=wm_dram[a : a + 128, :, :]
                    )
                    for cb in range(NCB):
                        c0 = cb * CBLK
                        xts = []
                        for i in range(K):
                            xt = xpool.tile(
                                [128, CBLK, WP], bf16, tag=f"xt{i}"
                            )
                            nc.sync.dma_start(
                                out=xt[:],
                                in_=xp[
                                    a + i : a + i + 128,
                                    c0 : c0 + CBLK,
                                    :,
                                ],
                            )
                            xts.append(xt)
                        for xh in range(NXH):
                            x0 = xh * XH

                            def wbc(t):
                                return (
                                    wt[:, t, x0 : x0 + XH]
                                    .unsqueeze(1)
                                    .broadcast_to((128, CBLK, XH))
                                )

                            if cb == NCB - 1:
                                # gpsimd owns this channel block: 2-pass
                                # mult/add with an fp32 SBUF accumulator
                                ga = gpool.tile(
                                    [128, CBLK, XH], f32, tag="ga"
                                )
                                for t in range(NT):
                                    i, j = t // K, t % K
                                    src = xts[i][:, :, x0 + j : x0 + j + XH]
                                    if t == 0:
                                        nc.gpsimd.tensor_tensor(
                                            out=ga[:],
                                            in0=src,
                                            in1=wbc(t),
                                            op=Alu.mult,
                                        )
                                    else:
                                        pg = gpool.tile(
                                            [128, CBLK, XH], bf16, tag="pg"
                                        )
                                        nc.gpsimd.tensor_tensor(
                                            out=pg[:],
                                            in0=src,
                                            in1=wbc(t),
                                            op=Alu.mult,
                                        )
                                        nc.gpsimd.tensor_tensor(
                                            out=ga[:],
                                            in0=ga[:],
                                            in1=pg[:],
                                            op=Alu.add,
                                        )
                                ot = opool.tile(
                                    [128, CBLK * XH], bf16, tag="ot"
                                )
                                nc.scalar.copy(
                                    out=ot[:],
                                    in_=ga.rearrange("p c x -> p (c x)"),
                                )
                            else:
                                acc = pspool.tile(
                                    [128, CBLK * XH], f32, tag="acc"
                                )
                                for t in range(NT):
                                    i, j = t // K, t % K
                                    p = ppool.tile(
                                        [128, CBLK, XH], bf16, tag="p"
                                    )
                                    nc.vector.tensor_tensor(
                                        out=p[:],
                                        in0=xts[i][:, :, x0 + j : x0 + j + XH],
                                        in1=wbc(t),
                                        op=Alu.mult,
                                    )
                                    pf = p.rearrange("p c x -> p (c x)")
                                    for s in range(CBLK * XH // 512):
                                        nc.tensor.matmul(
                                            acc[:, s * 512 : (s + 1) * 512],
                                            ident_sb[:],
                                            pf[:, s * 512 : (s + 1) * 512],
                                            start=(t == 0),
                                            stop=(t == NT - 1),
                                        )
                                ot = opool.tile(
                                    [128, CBLK * XH], bf16, tag="ot"
                                )
                                nc.scalar.copy(out=ot[:], in_=acc[:])
                            nc.sync.dma_start(
                                out=y_out[
                                    a : a + 128, c0 : c0 + CBLK, x0 : x0 + XH
                                ],
                                in_=ot.rearrange("p (c x) -> p c x", c=CBLK),
                            )
                        if a == 0 and cb < 6:
                            # slip chunk 1's wm build under chunk 0's conv
                            n3 = 3 if cb < 4 else 2
                            for k3 in range(n3):
                                g = GPC + (3 * cb + k3 if cb < 4 else 12 + 2 * (cb - 4) + k3)
                                wm_group(g)

    nc.finalize()
    return nc


def _get_nc():
    if "nc" not in _CACHE:
        _CACHE["nc"] = _build_nc()
    return _CACHE["nc"]


def _make_in_maps(inputs):
    import concourse.mybir as mybir

    bf16 = mybir.dt.np(mybir.dt.bfloat16)

    x = np.ascontiguousarray(inputs["input"], dtype=np.float32)
    # pad spatially, then [b, row, c, x] channel-mid layout
    xpad = np.pad(x, ((0, 0), (0, 0), (PAD, PAD), (PAD, PAD)))
    xpad = np.ascontiguousarray(xpad.transpose(0, 2, 1, 3)).astype(bf16)
    bkf = np.ascontiguousarray(inputs["buckets"], dtype=np.int32).astype(
        np.float32
    ).astype(bf16)  # ids < 64: exact in bf16
    bank2 = (
        np.ascontiguousarray(inputs["kernel_bank"], dtype=np.float32)
        .reshape(NB, NT)
        .astype(bf16)
    )
    iota64 = np.arange(NB, dtype=np.float32).reshape(NB, 1)
    ident = np.eye(128, dtype=np.float32).astype(bf16)
    return [
        {
            "xp": xpad[i],
            "bkf": bkf[i],
            "bank": bank2,
            "iota": iota64,
            "ident": ident,
        }
        for i in range(N_CORES)
    ]


def kernel(input, kernel_bank, buckets):
    from concourse.bass_utils import run_bass_kernel_spmd

    nc = _get_nc()
    in_maps = _make_in_maps(
        {"input": input, "kernel_bank": kernel_bank, "buckets": buckets}
    )
    res = run_bass_kernel_spmd(nc, in_maps, list(range(N_CORES)))
    # device output is [H, C, W] bf16; back to [C, H, W] fp32
    out = np.stack(
        [
            res.results[i]["y"].astype(np.float32).transpose(1, 0, 2)
            for i in range(N_CORES)
        ],
        axis=0,
    )
    return np.ascontiguousarray(out, dtype=np.float32)


# revision 9
# speedup vs baseline: 1.2301x; 1.1949x over previous
"""Bucket-indexed spatially-varying (channel-shared) 5x5 convolution on 8 trn2 cores.

out[b,c,y,x] = sum_{i,j} pad(input)[b,c,y+i,x+j] * kernel_bank[buckets[b,y,x], i, j]

Strategy (data-parallel over batch, one image per core), all bf16 on device:
  * Phase A: buckets -> one-hot (DVE is_equal) -> PE matmul against the
    [64,25] bank -> per-pixel weight map wm staged to DRAM as [y, tap, x].
  * Phase B layout: partition = output row y (2 chunks of 128), free =
    (c, x) with x minor. Five row-shifted copies of the input tile make
    the dy shift a partition-aligned read; dx is a free-dim offset.
  * Per tap: ONE big DVE tensor_tensor mult (weight map broadcast across
    c via a stride-0 AP; x-minor keeps every operand packed bf16 so the
    DVE 2x perf mode engages):
        P[y, c, x] = xs[y+i, c, x+j] * wm[y, t, x]
    The 25 tap products are accumulated on the otherwise-idle PE with
    identity-stationary matmuls into PSUM (fp32), then evicted to bf16
    by the ACT engine. Host converts bf16 -> fp32.
"""

import sys

sys.path.insert(0, "/opt/trn_rl_repo")

import numpy as np

B, C, H, W = 8, 128, 256, 256
K, NB = 5, 64
PAD = (K - 1) // 2  # 2
HP, WP = H + 2 * PAD, W + 2 * PAD  # 260, 260
N_CORES = 8
NT = K * K  # 25 taps

CBLK = 16  # channel block
NCB = C // CBLK  # 8
XH = 128  # x half width
NXH = W // XH  # 2
GPT = 4  # taps whose products run on gpsimd

_CACHE = {}


def _build_nc():
    import concourse.bacc as bacc
    import concourse.mybir as mybir
    from concourse import tile

    f32 = mybir.dt.float32
    bf16 = mybir.dt.bfloat16
    Alu = mybir.AluOpType
    Act = mybir.ActivationFunctionType

    nc = bacc.Bacc(None)

    # channel-mid layout [row, c, x]: per-partition contiguous c-block rows
    xp = nc.dram_tensor("xp", [HP, C, WP], bf16, kind="ExternalInput")
    bkf = nc.dram_tensor("bkf", [H, W], bf16, kind="ExternalInput")
    bank = nc.dram_tensor("bank", [NB, NT], bf16, kind="ExternalInput")
    iota = nc.dram_tensor("iota", [NB, 1], f32, kind="ExternalInput")
    ident = nc.dram_tensor("ident", [128, 128], bf16, kind="ExternalInput")
    y_out = nc.dram_tensor("y", [H, C, W], bf16, kind="ExternalOutput")

    GROWS = 8  # bucket rows per wm-build group
    GPIX = GROWS * W  # 2048
    GPC = 128 // GROWS  # 16 groups per y chunk

    with tile.TileContext(nc) as tc:
        with tc.tile_pool(name="dram", bufs=1, space="DRAM") as dpool:
            # weight map staged in DRAM as [y, tap, x]
            wm_dram = dpool.tile([H, NT, W], bf16)

            with (
                tc.tile_pool(name="const", bufs=1) as kpool,
                tc.tile_pool(name="wbuild", bufs=2) as wpool,
                tc.tile_pool(name="wm", bufs=2) as wmpool,
                tc.tile_pool(name="xs", bufs=2) as xpool,
                tc.tile_pool(name="prod", bufs=4) as ppool,
                tc.tile_pool(name="gacc", bufs=2) as gpool,
                tc.tile_pool(name="out", bufs=2) as opool,
                tc.tile_pool(name="psum", bufs=2, space="PSUM") as pspool,
            ):
                bank_sb = kpool.tile([NB, NT], bf16)
                nc.sync.dma_start(out=bank_sb[:], in_=bank[:])
                iota_sb = kpool.tile([NB, 1], f32)
                nc.sync.dma_start(out=iota_sb[:], in_=iota[:])
                ident_sb = kpool.tile([128, 128], bf16)
                nc.sync.dma_start(out=ident_sb[:], in_=ident[:])

                def wm_group(g):
                    # one-hot + PE matmul against the bank -> wm rows
                    brep = wpool.tile([NB, GPIX], bf16, tag="brep")
                    nc.sync.dma_start(
                        out=brep[:],
                        in_=bkf[g * GROWS : (g + 1) * GROWS, :]
                        .rearrange("(o h) w -> o (h w)", o=1)
                        .broadcast_to((NB, GPIX)),
                    )
                    oh = wpool.tile([NB, GPIX], bf16, tag="oh")
                    nc.vector.tensor_scalar(
                        out=oh[:],
                        in0=brep[:],
                        scalar1=iota_sb[:],
                        scalar2=None,
                        op0=Alu.is_equal,
                    )
                    # borrow the conv psum buffers (same tag/shape)
                    ps = pspool.tile([128, CBLK * XH], f32, tag="acc")
                    for s in range(GPIX // 512):
                        nc.tensor.matmul(
                            ps[0:NT, s * 512 : (s + 1) * 512],
                            bank_sb[:],
                            oh[:, s * 512 : (s + 1) * 512],
                            start=True,
                            stop=True,
                        )
                    wms = wpool.tile([NT, GPIX], bf16, tag="wms")
                    nc.scalar.copy(out=wms[:], in_=ps[0:NT, 0:GPIX])
                    y0 = g * GROWS
                    # keep the SBUF partition dim (t) first on both sides;
                    # an SBUF-side rearrange that moves the partition dim
                    # scrambles the transfer.
                    nc.sync.dma_start(
                        out=wm_dram[y0 : y0 + GROWS, :, :].rearrange(
                            "y t x -> t y x"
                        ),
                        in_=wms.rearrange("t (y x) -> t y x", y=GROWS),
                    )

                # chunk 0's weight map as a prefix; chunk 1's is
                # interleaved into chunk 0's conv loop below
                for g in range(GPC):
                    wm_group(g)

                for a in (0, 128):  # y chunk
                    wt = wmpool.tile([128, NT, W], bf16, tag="wt")
                    nc.sync.dma_start(
                        out=wt[:], in_=wm_dram[a : a + 128, :, :]
                    )
                    for cb in range(NCB):
                        c0 = cb * CBLK
                        xts = []
                        for i in range(K):
                            xt = xpool.tile(
                                [128, CBLK, WP], bf16, tag=f"xt{i}"
                            )
                            nc.sync.dma_start(
                                out=xt[:],
                                in_=xp[
                                    a + i : a + i + 128,
                                    c0 : c0 + CBLK,
                                    :,
                                ],
                            )
                            xts.append(xt)
                        for xh in range(NXH):
                            x0 = xh * XH

                            def wbc(t):
                                return (
                                    wt[:, t, x0 : x0 + XH]
                                    .unsqueeze(1)
                                    .broadcast_to((128, CBLK, XH))
                                )

                            acc = pspool.tile(
                                [128, CBLK * XH], f32, tag="acc"
                            )
                            # gpsimd computes the last GPT taps' products
                            # concurrently (emitted first so it starts
                            # early; its matmuls drain last on the PE)
                            gps = []
                            for t in range(NT - GPT, NT):
                                i, j = t // K, t % K
                                pg = gpool.tile(
                                    [128, CBLK, XH], bf16, tag=f"pg{t % GPT}"
                                )
                                nc.gpsimd.tensor_tensor(
                                    out=pg[:],
                                    in0=xts[i][:, :, x0 + j : x0 + j + XH],
                                    in1=wbc(t),
                                    op=Alu.mult,
                                )
                                gps.append(pg)
                            for t in range(NT - GPT):
                                i, j = t // K, t % K
                                p = ppool.tile(
                                    [128, CBLK, XH], bf16, tag="p"
                                )
                                nc.vector.tensor_tensor(
                                    out=p[:],
                                    in0=xts[i][:, :, x0 + j : x0 + j + XH],
                                    in1=wbc(t),
                                    op=Alu.mult,
                                )
                                pf = p.rearrange("p c x -> p (c x)")
                                for s in range(CBLK * XH // 512):
                                    nc.tensor.matmul(
                                        acc[:, s * 512 : (s + 1) * 512],
                                        ident_sb[:],
                                        pf[:, s * 512 : (s + 1) * 512],
                                        start=(t == 0),
                                        stop=False,
                                    )
                            for k4, pg in enumerate(gps):
                                pf = pg.rearrange("p c x -> p (c x)")
                                for s in range(CBLK * XH // 512):
                                    nc.tensor.matmul(
                                        acc[:, s * 512 : (s + 1) * 512],
                                        ident_sb[:],
                                        pf[:, s * 512 : (s + 1) * 512],
                                        start=False,
                                        stop=(k4 == GPT - 1),
                                    )
                            ot = opool.tile(
                                [128, CBLK * XH], bf16, tag="ot"
                            )
                            nc.scalar.copy(out=ot[:], in_=acc[:])
                            nc.sync.dma_start(
                                out=y_out[
                                    a : a + 128, c0 : c0 + CBLK, x0 : x0 + XH
                                ],
                                in_=ot.rearrange("p (c x) -> p c x", c=CBLK),
                            )
                        if a == 0 and cb < 6:
                            # slip chunk 1's wm build under chunk 0's conv
                            n3 = 3 if cb < 4 else 2
                            for k3 in range(n3):
                                g = GPC + (3 * cb + k3 if cb < 4 else 12 + 2 * (cb - 4) + k3)
                                wm_group(g)

    nc.finalize()
    return nc


def _get_nc():
    if "nc" not in _CACHE:
        _CACHE["nc"] = _build_nc()
    return _CACHE["nc"]


def _make_in_maps(inputs):
    import concourse.mybir as mybir

    bf16 = mybir.dt.np(mybir.dt.bfloat16)

    x = np.ascontiguousarray(inputs["input"], dtype=np.float32)
    # pad spatially, then [b, row, c, x] channel-mid layout
    xpad = np.pad(x, ((0, 0), (0, 0), (PAD, PAD), (PAD, PAD)))
    xpad = np.ascontiguousarray(xpad.transpose(0, 2, 1, 3)).astype(bf16)
    bkf = np.ascontiguousarray(inputs["buckets"], dtype=np.int32).astype(
        np.float32
    ).astype(bf16)  # ids < 64: exact in bf16
    bank2 = (
        np.ascontiguousarray(inputs["kernel_bank"], dtype=np.float32)
        .reshape(NB, NT)
        .astype(bf16)
    )
    iota64 = np.arange(NB, dtype=np.float32).reshape(NB, 1)
    ident = np.eye(128, dtype=np.float32).astype(bf16)
    return [
        {
            "xp": xpad[i],
            "bkf": bkf[i],
            "bank": bank2,
            "iota": iota64,
            "ident": ident,
        }
        for i in range(N_CORES)
    ]


def kernel(input, kernel_bank, buckets):
    from concourse.bass_utils import run_bass_kernel_spmd

    nc = _get_nc()
    in_maps = _make_in_maps(
        {"input": input, "kernel_bank": kernel_bank, "buckets": buckets}
    )
    res = run_bass_kernel_spmd(nc, in_maps, list(range(N_CORES)))
    # device output is [H, C, W] bf16; back to [C, H, W] fp32
    out = np.stack(
        [
            res.results[i]["y"].astype(np.float32).transpose(1, 0, 2)
            for i in range(N_CORES)
        ],
        axis=0,
    )
    return np.ascontiguousarray(out, dtype=np.float32)
